# revision 15
# baseline (speedup 1.0000x reference)
"""ChebyKAN linear layer on 8 Trainium2 NeuronCores.

Computation: out[b,o] = sum_{i,d} T_d(tanh(x[b,i])) * coef[i,o,d]
  == sum_d T_d(tanh(x)) @ C_d   (9 accumulated 8192x1024x1024 matmuls)

Strategy:
  - Data-parallel over batch: core c handles rows [c*1024, (c+1)*1024).
  - Host pre-transposes each core's x slice to (in_features, batch) layout so
    the contraction dim (i) lands on SBUF partitions, and repacks the
    coefficients to (d, i, o) bf16.
  - On-chip: ACT computes tanh in fp32, DVE runs the Chebyshev recursion
    T_d = 2 t T_{d-1} - T_{d-2} in fp32 (scalar_tensor_tensor fuses the
    2*t*T_{d-1} product into one op), ACT casts each T_d to bf16, and PE
    accumulates the 8 degree-matmuls (d=1..8) in fp32 PSUM.
  - The d=0 term (T_0 == 1) is folded on the host into a single extra
    128-contraction "bias" matmul: W_bias[k,o] = sum_j C_0[j*128+k, o],
    multiplied by an all-ones stationary tile.
  - Per core the 1024-row batch is processed in two 512-column halves; each
    half keeps its full output (4 b-chunks x 2 o-halves) resident in all
    8 PSUM banks while 65 k-blocks accumulate into it.

Numerics (validated on HW): rel l2 error vs fp32 reference ~2e-3.

Performance (8-core SPMD, measured via on-device For_i loop slope because
the axon tunnel's ~80 ms RPC overhead hides the kernel and NTFF profiling
is unavailable through it): ~275-285 us per full (8192, 1024) evaluation.
Budget: 1056 N=512 matmuls/core = 228 us PE floor; +~27 us serialized
LDWEIGHTS (walrus --enable-ldw-opt is hardcoded false and =true fails
codegen, so no background-buffer overlap; a post-compile pass below elides
the 524 redundant back-to-back reloads, worth ~7 us); remainder is Tile
per-instruction semaphore cost and half-boundary PSUM drains.
"""

import numpy as np
import ml_dtypes

BATCH = 8192
IN_F = 1024
OUT_F = 1024
DEG = 8  # degree; DEG+1 coefficients per (i,o)
N_CORES = 8
B_CORE = BATCH // N_CORES  # 1024
P = 128
HALF = 512  # batch columns processed per PSUM-resident output block
NI = IN_F // P  # 8 contraction tiles
NBC = HALF // P  # 4 b-chunks per half
NOH = OUT_F // 512  # 2 output halves of 512
N_HALF = B_CORE // HALF  # 2

_CACHED_NC = {}


def _build_bass(loop_r=None, variant=""):
    """Build the Bass program. loop_r wraps the whole compute in a hardware
    For loop of loop_r iterations (benchmark-only; slope over loop_r gives
    per-iteration HW time since the axon RPC overhead is per-call)."""
    import contextlib

    import concourse.mybir as mybir
    import concourse.tile as tile
    from concourse import bacc

    f32 = mybir.dt.float32
    bf16 = mybir.dt.bfloat16
    mult = mybir.AluOpType.mult
    sub = mybir.AluOpType.subtract
    Tanh = mybir.ActivationFunctionType.Tanh

    import json as _json

    def _dedup_ldweights(b):
        """Remove InstLdweights that reload the identical stationary operand
        still held by the PE array. Tile emits one Ldweights per matmul, so a
        weight reused by consecutive matmuls is loaded twice; each redundant
        load costs ~55 ns of serial PE time. Only sync-free exact duplicates
        are removed. Instructions on OTHER engine queues interleaved in the
        block's program order cannot disturb the PE weight registers, so they
        do not reset the tracked key; the only PE-queue instructions between
        a duplicate pair are Matmult (does not disturb weights) and sync-free
        EventSemaphore. An EventSemaphore with a wait is treated as a fence
        (conservative: it could order an SBUF rewrite of the stationary)."""
        n_removed = 0
        PE = mybir.EngineType.PE
        for fn in b.m.functions:
            for blk in fn.blocks:
                last_key = None
                keep = []
                for inst in blk.instructions:
                    if isinstance(inst, mybir.InstLdweights):
                        d = _json.loads(
                            mybir.instruction_to_pretty_json_string(inst)
                        )
                        si = d.get("sync_info") or {}
                        has_sync = bool(
                            si.get("on_wait") or si.get("on_update")
                        )
                        key = _json.dumps(
                            [
                                d.get("ins"),
                                d.get("perf_mode"),
                                d.get("is_transpose"),
                                d.get("tile_position"),
                                d.get("tile_size"),
                            ],
                            sort_keys=True,
                        )
                        if key == last_key and not has_sync:
                            n_removed += 1
                            continue
                        last_key = key
                    elif inst.engine != PE:
                        pass  # other-engine work; PE array unaffected
                    elif isinstance(inst, mybir.InstMatmult):
                        pass  # matmult does not disturb loaded weights
                    elif isinstance(inst, mybir.InstEventSemaphore):
                        si = inst.sync_info
                        if si is not None and si.on_wait:
                            # conservative fence: a PE-queue wait could
                            # order an SBUF rewrite of the stationary
                            last_key = None
                    else:
                        last_key = None
                    keep.append(inst)
                blk.instructions[:] = keep
        b._ldw_removed = n_removed

    class _Bacc(bacc.Bacc):
        def compile(self):
            super().compile()
            _dedup_ldweights(self)

    nc = _Bacc(name="chebykan")
    xt = nc.dram_tensor("xt", (IN_F, B_CORE), f32, kind="ExternalInput")
    w = nc.dram_tensor("w", (DEG, IN_F, OUT_F), bf16, kind="ExternalInput")
    brep = nc.dram_tensor("brep", (P, OUT_F), f32, kind="ExternalInput")
    out = nc.dram_tensor("out", (B_CORE, OUT_F), f32, kind="ExternalOutput")

    with (
        tile.TileContext(nc) as tc,
        tc.tile_pool(name="wpool", bufs=12) as wpool,
        tc.tile_pool(name="xpool", bufs=8) as xpool,
        tc.tile_pool(name="tanh", bufs=3) as tanpool,
        tc.tile_pool(name="rec", bufs=6) as rpool,
        tc.tile_pool(name="ch", bufs=16) as chpool,
        tc.tile_pool(name="const", bufs=1) as cpool,
        tc.tile_pool(name="outp", bufs=8) as opool,
        tc.tile_pool(name="psum", bufs=1, space="PSUM") as pspool,
    ):
        ones = cpool.tile([P, P], bf16)
        nc.vector.memset(ones[:], 1.0)
        # d=0 bias, pre-replicated across partitions on the host; added
        # during the PSUM drain instead of spending PE matmuls on it
        biasrep = cpool.tile([P, OUT_F], f32)
        nc.sync.dma_start(biasrep[:], brep[:, :])

        loop_cm = (
            tc.For_i(
                0,
                loop_r,
                1,
                hint_engines=(mybir.EngineType.PE, mybir.EngineType.SP),
            )
            if loop_r is not None
            else contextlib.nullcontext()
        )
        with loop_cm:
            _emit_body(nc, tc, xt, w, out, ones, biasrep,
                       wpool, xpool, tanpool, rpool, chpool, opool, pspool,
                       f32, bf16, mult, sub, Tanh, variant)
    nc.finalize()
    return nc


def _emit_body(nc, tc, xt, w, out, ones, biasrep,
               wpool, xpool, tanpool, rpool, chpool, opool, pspool,
               f32, bf16, mult, sub, Tanh, variant=""):
    import concourse.mybir as mybir
    add = mybir.AluOpType.add
    wide = variant == "wide"
    n_oh = 1 if variant == "halfmm" else NOH
    if variant in ("mmonly", "mmrows256"):
        # diagnostic: pure PE stream — same matmult/psum-group structure as
        # the real kernel but constant operands, no DMA/recursion/drains.
        # Measures sustained matmult throughput on this hardware.
        nfree = 256 if variant == "mmrows256" else 512
        wt0 = wpool.tile([P, OUT_F], bf16, tag="w")
        nc.sync.dma_start(wt0[:], w[0, 0:P, :])
        for h in range(N_HALF):
            ps = [
                [
                    pspool.tile([P, nfree], f32, tag=f"psm_{bc}_{oh}",
                                name=f"psm_{bc}_{oh}")
                    for oh in range(NOH)
                ]
                for bc in range(NBC)
            ]
            for i in range(NI):
                for d in range(1, DEG + 1):
                    start = i == 0 and d == 1
                    stop = i == NI - 1 and d == DEG
                    for bc in range(NBC):
                        for oh in range(NOH):
                            nc.tensor.matmul(
                                ps[bc][oh],
                                ones,
                                wt0[:, oh * nfree : (oh + 1) * nfree],
                                start=start,
                                stop=stop,
                            )
        return
    for h in range(N_HALF):
            if wide:
                # one [P, 1024] tile spans 2 PSUM banks; a single matmult
                # accumulates the full 1024-wide output row block
                ps = [
                    pspool.tile([P, OUT_F], f32, tag=f"psw_{bc}",
                                name=f"psw_{bc}")
                    for bc in range(NBC)
                ]
            else:
                ps = [
                    [
                        pspool.tile(
                            [P, 512], f32, tag=f"ps_{bc}_{oh}",
                            name=f"ps_{bc}_{oh}"
                        )
                        for oh in range(n_oh)
                    ]
                    for bc in range(NBC)
                ]
            bm = variant == "bm"
            for i in range(NI):
                last_i = i == NI - 1
                chds = {}
                wts = {}
                xti = xpool.tile([P, HALF], f32, tag="x")
                nc.sync.dma_start(
                    xti[:], xt[i * P : (i + 1) * P, h * HALF : (h + 1) * HALF]
                )
                t = tanpool.tile([P, HALF], f32, tag="t")
                nc.scalar.activation(t[:], xti[:], Tanh)

                tm2 = None  # T_{d-2} (fp32); None encodes T_0 == 1
                tm1 = t  # T_{d-1} (fp32)
                ch1 = None
                for d in range(1, DEG + 1):
                    last = d == DEG
                    if variant == "norec" and d > 1:
                        chd = ch1
                    else:
                        chd = chpool.tile([P, HALF], bf16, tag="ch")
                    if d == 1:
                        nc.scalar.copy(chd[:], t[:])
                        ch1 = chd
                        cur = t
                    elif variant == "norec":
                        cur = None
                    else:
                        # pr = (T_{d-1} * 2) * t  (one fused DVE op)
                        pr = rpool.tile([P, HALF], f32, tag="rec")
                        nc.vector.scalar_tensor_tensor(
                            pr[:], tm1[:], 2.0, t[:], mult, mult
                        )
                        if d == 2:
                            # T_2 = pr - 1
                            cur = rpool.tile([P, HALF], f32, tag="rec")
                            nc.vector.tensor_scalar_sub(cur[:], pr[:], 1.0)
                            nc.scalar.copy(chd[:], cur[:])
                        elif not last:
                            cur = rpool.tile([P, HALF], f32, tag="rec")
                            nc.vector.tensor_tensor(cur[:], pr[:], tm2[:], sub)
                            nc.scalar.copy(chd[:], cur[:])
                        else:
                            # final degree: write the bf16 tile directly
                            cur = None
                            nc.vector.tensor_tensor(chd[:], pr[:], tm2[:], sub)
                    tm2, tm1 = tm1, cur

                    if variant == "nodma":
                        if i == 0 and d == 1:
                            wt0 = wpool.tile([P, 1, OUT_F], bf16, tag="w")
                            nc.sync.dma_start(wt0[:, 0], w[0, 0:P, :])
                        wt = wt0[:, 0]
                    else:
                        wt = wpool.tile([P, OUT_F], bf16, tag="w")
                        nc.sync.dma_start(wt[:], w[d - 1, i * P : (i + 1) * P, :])
                    stop = i == NI - 1 and d == DEG
                    start = i == 0 and d == 1
                    for bc in range(NBC):
                        if variant == "oneldw":
                            # diagnostic: constant stationary everywhere so
                            # the post-compile dedup strips nearly every
                            # ldweights; numerics wrong, timing isolates the
                            # ldweights contribution
                            lhsT = ones[:]
                        else:
                            lhsT = chd[:, bc * P : (bc + 1) * P]
                        if wide:
                            nc.tensor.matmul(
                                ps[bc], lhsT, wt[:], start=start, stop=stop
                            )
                        else:
                            for oh in range(n_oh):
                                nc.tensor.matmul(
                                    ps[bc][oh],
                                    lhsT,
                                    wt[:, oh * 512 : (oh + 1) * 512],
                                    start=start,
                                    stop=stop,
                                )
            # Drain this half's PSUM to SBUF (adding the d=0 bias) and then
            # HBM. The bias-add rides the drain copy for free on DVE.
            if variant == "nodrain":
                continue
            for bc in range(NBC):
                for oh in range(n_oh):
                    ot = opool.tile([P, 512], f32, tag="ot")
                    src = (
                        ps[bc][:, oh * 512 : (oh + 1) * 512]
                        if wide
                        else ps[bc][oh]
                    )
                    nc.vector.tensor_tensor(
                        ot[:], src, biasrep[:, oh * 512 : (oh + 1) * 512], add
                    )
                    r0 = h * HALF + bc * P
                    nc.sync.dma_start(
                        out[r0 : r0 + P, oh * 512 : (oh + 1) * 512], ot[:]
                    )


def _get_nc(loop_r=None, variant=""):
    key = (loop_r, variant)
    if key not in _CACHED_NC:
        _CACHED_NC[key] = _build_bass(loop_r, variant)
    return _CACHED_NC[key]


def _prep_inputs(x, coefficients):
    bf16 = ml_dtypes.bfloat16
    x = np.asarray(x, dtype=np.float32)
    coef = np.asarray(coefficients, dtype=np.float32)
    # (d, i, o) bf16 for d = 1..DEG
    w_all = np.ascontiguousarray(coef.transpose(2, 0, 1)[1 : DEG + 1]).astype(bf16)
    # d=0 term is a per-output bias (T_0 == 1): summed over i on the host,
    # replicated across the 128 partitions, added during the PSUM drain
    bias = coef[:, :, 0].sum(axis=0, dtype=np.float64).astype(np.float32)
    brep_arr = np.ascontiguousarray(
        np.broadcast_to(bias, (P, OUT_F)), dtype=np.float32
    )
    in_maps = []
    for c in range(N_CORES):
        xc = x[c * B_CORE : (c + 1) * B_CORE, :]
        in_maps.append(
            {
                "xt": np.ascontiguousarray(xc.T),
                "w": w_all,
                "brep": brep_arr,
            }
        )
    return in_maps


VARIANT = ""  # production variant used by kernel()/run()


def run(x, coefficients, trace=False, tmpdir=None):
    """Run on hardware; returns (out, BassKernelResults)."""
    from concourse.bass_utils import run_bass_kernel_spmd

    nc = _get_nc(None, VARIANT)
    in_maps = _prep_inputs(x, coefficients)
    res = run_bass_kernel_spmd(
        nc,
        in_maps,
        core_ids=list(range(N_CORES)),
        trace=trace,
        tmpdir=tmpdir,
    )
    out = np.concatenate([r["out"] for r in res.results], axis=0)
    return np.ascontiguousarray(out, dtype=np.float32), res


def kernel(x, coefficients):
    out, _ = run(x, coefficients, trace=False)
    return out



# revision 27
# speedup vs baseline: 1.1190x; 1.1190x over previous
"""ChebyKAN linear layer on 8 Trainium2 NeuronCores.

Computation: out[b,o] = sum_{i,d} T_d(tanh(x[b,i])) * coef[i,o,d]
  == sum_d T_d(tanh(x)) @ C_d   (9 accumulated 8192x1024x1024 matmuls)

Strategy:
  - Data-parallel over batch: core c handles rows [c*1024, (c+1)*1024).
  - Host pre-transposes each core's x slice to (in_features, batch) layout so
    the contraction dim (i) lands on SBUF partitions, and repacks the
    coefficients to (d, i, o): bf16 for degrees 1..6, fp8 e4m3 (scaled by
    2^12, k-tile pairs on a separate axis) for degrees 7..8.
  - On-chip: ACT computes tanh in fp32, DVE runs the Chebyshev recursion
    T_d = 2 t T_{d-1} - T_{d-2} in fp32 (scalar_tensor_tensor fuses the
    2*t*T_{d-1} product into one op), ACT casts T_1..T_6 to bf16 and
    T_7..T_8 to fp8 e4m3 DoubleRow pair tiles [128, 2, 512].
  - PE accumulates per half in two PSUM waves: wave 1 = degrees 1..6 in
    bf16 (start at (i=0,d=1)); a DVE drain captures wave 1 + the d=0 bias
    (host-precomputed, fp32, partition-replicated) into SBUF; wave 2 =
    degrees 7..8 as fp8 DoubleRow matmuls (2 k-tiles per instruction, 2x
    MAC rate); the final DVE merge computes wave2 * 2^-12 + wave1 and DMAs
    out. Splitting waves keeps the 2^12 fp8 coefficient scale out of the
    bf16 accumulation.
  - The d=0 term rides the drain (no PE matmuls for it).
  - Per core the 1024-row batch is processed in two 512-column halves; each
    half keeps its full output (4 b-chunks x 2 o-halves) resident in all
    8 PSUM banks while the k-blocks accumulate into it.

Numerics (validated on HW): rel l2 error vs fp32 reference ~1.5e-2
(threshold 2e-2; fp8 on 2 of 8 degrees contributes ~sqrt(2)*1.04e-2, the
bf16 remainder ~1.8e-3).

Performance notes (8-core SPMD, measured via on-device For_i loop slope
because the axon tunnel's ~80-95 ms RPC overhead hides the kernel and NTFF
profiling is unavailable through it; cross-process slope noise is +-5 us):
  - Measured matmult throughput on this part (mmonly/mmrows256 diagnostic
    variants, pure PE stream, no deps) is 0.51 ns/row — 1.96 GHz effective,
    NOT the 2.4 GHz nominal — and is linear in rows with ~zero per-
    instruction overhead. The all-bf16 kernel (1024 512-row matmuls,
    267 us of stream) measured 269-283 us: already ~97% of the PE roofline.
  - Ldweights are nearly free when sync-free (hidden in the matmul
    pipeline): deduping 187 extra reloads moved the time < 2 us. The
    engine-aware dedup pass below keeps them at the 1-per-(bc-chunk) floor.
  - DMA (weights re-streamed per half, 33.6 MB/iter/core), PSUM drains and
    half boundaries are all second-order (nodma/nodrain variants within
    noise of base).
  - fp8 DoubleRow on degrees 7..8 removes 1/4 of the bf16 rows.
"""

import numpy as np
import ml_dtypes

BATCH = 8192
IN_F = 1024
OUT_F = 1024
DEG = 8  # degree; DEG+1 coefficients per (i,o)
N_CORES = 8
B_CORE = BATCH // N_CORES  # 1024
P = 128
HALF = 512  # batch columns processed per PSUM-resident output block
NI = IN_F // P  # 8 contraction tiles
NBC = HALF // P  # 4 b-chunks per half
NOH = OUT_F // 512  # 2 output halves of 512
N_HALF = B_CORE // HALF  # 2

_CACHED_NC = {}


def _build_bass(loop_r=None, variant=""):
    """Build the Bass program. loop_r wraps the whole compute in a hardware
    For loop of loop_r iterations (benchmark-only; slope over loop_r gives
    per-iteration HW time since the axon RPC overhead is per-call)."""
    import contextlib

    import concourse.mybir as mybir
    import concourse.tile as tile
    from concourse import bacc

    f32 = mybir.dt.float32
    bf16 = mybir.dt.bfloat16
    mult = mybir.AluOpType.mult
    sub = mybir.AluOpType.subtract
    Tanh = mybir.ActivationFunctionType.Tanh

    import json as _json

    def _dedup_ldweights(b):
        """Remove InstLdweights that reload the identical stationary operand
        still held by the PE array. Tile emits one Ldweights per matmul, so a
        weight reused by consecutive matmuls is loaded twice; each redundant
        load costs ~55 ns of serial PE time. Only sync-free exact duplicates
        are removed. Instructions on OTHER engine queues interleaved in the
        block's program order cannot disturb the PE weight registers, so they
        do not reset the tracked key; the only PE-queue instructions between
        a duplicate pair are Matmult (does not disturb weights) and sync-free
        EventSemaphore. An EventSemaphore with a wait is treated as a fence
        (conservative: it could order an SBUF rewrite of the stationary)."""
        n_removed = 0
        PE = mybir.EngineType.PE
        for fn in b.m.functions:
            for blk in fn.blocks:
                last_key = None
                keep = []
                for inst in blk.instructions:
                    if isinstance(inst, mybir.InstLdweights):
                        d = _json.loads(
                            mybir.instruction_to_pretty_json_string(inst)
                        )
                        si = d.get("sync_info") or {}
                        has_sync = bool(
                            si.get("on_wait") or si.get("on_update")
                        )
                        key = _json.dumps(
                            [
                                d.get("ins"),
                                d.get("perf_mode"),
                                d.get("is_transpose"),
                                d.get("tile_position"),
                                d.get("tile_size"),
                            ],
                            sort_keys=True,
                        )
                        if key == last_key and not has_sync:
                            n_removed += 1
                            continue
                        last_key = key
                    elif inst.engine != PE:
                        pass  # other-engine work; PE array unaffected
                    elif isinstance(inst, mybir.InstMatmult):
                        pass  # matmult does not disturb loaded weights
                    elif isinstance(inst, mybir.InstEventSemaphore):
                        si = inst.sync_info
                        if si is not None and si.on_wait:
                            # conservative fence: a PE-queue wait could
                            # order an SBUF rewrite of the stationary
                            last_key = None
                    else:
                        last_key = None
                    keep.append(inst)
                blk.instructions[:] = keep
        b._ldw_removed = n_removed

    class _Bacc(bacc.Bacc):
        def compile(self):
            super().compile()
            _dedup_ldweights(self)

    f8 = mybir.dt.float8e4

    nc = _Bacc(name="chebykan")
    xt = nc.dram_tensor("xt", (IN_F, B_CORE), f32, kind="ExternalInput")
    w = nc.dram_tensor("w", (DEG, IN_F, OUT_F), bf16, kind="ExternalInput")
    # degrees 7..8 as fp8 e4m3, k-tile pairs interleaved for DoubleRow:
    # w8[dd, j, k, s, o] = coef[(2j+s)*128 + k, o, 7+dd] * 2^12
    w8 = nc.dram_tensor(
        "w8", (2, NI // 2, P, 2, OUT_F), f8, kind="ExternalInput"
    )
    brep = nc.dram_tensor("brep", (P, OUT_F), f32, kind="ExternalInput")
    out = nc.dram_tensor("out", (B_CORE, OUT_F), f32, kind="ExternalOutput")

    with (
        tile.TileContext(nc) as tc,
        tc.tile_pool(name="wpool", bufs=12) as wpool,
        tc.tile_pool(name="w8pool", bufs=6) as w8pool,
        tc.tile_pool(name="xpool", bufs=8) as xpool,
        tc.tile_pool(name="tanh", bufs=3) as tanpool,
        tc.tile_pool(name="rec", bufs=6) as rpool,
        tc.tile_pool(name="ch", bufs=16) as chpool,
        tc.tile_pool(name="ch8", bufs=10) as ch8pool,
        tc.tile_pool(name="const", bufs=1) as cpool,
        tc.tile_pool(name="outp", bufs=16) as opool,
        tc.tile_pool(name="psum", bufs=1, space="PSUM") as pspool,
    ):
        ones = cpool.tile([P, P], bf16)
        nc.vector.memset(ones[:], 1.0)
        # d=0 bias, pre-replicated across partitions on the host; added
        # during the PSUM drain instead of spending PE matmuls on it
        biasrep = cpool.tile([P, OUT_F], f32)
        nc.sync.dma_start(biasrep[:], brep[:, :])

        loop_cm = (
            tc.For_i(
                0,
                loop_r,
                1,
                hint_engines=(mybir.EngineType.PE, mybir.EngineType.SP),
            )
            if loop_r is not None
            else contextlib.nullcontext()
        )
        with loop_cm:
            _emit_body(nc, tc, xt, w, w8, out, ones, biasrep,
                       wpool, w8pool, xpool, tanpool, rpool, chpool, ch8pool,
                       opool, pspool, f32, bf16, f8, mult, sub, Tanh, variant)
    nc.finalize()
    return nc


def _emit_body(nc, tc, xt, w, w8, out, ones, biasrep,
               wpool, w8pool, xpool, tanpool, rpool, chpool, ch8pool,
               opool, pspool, f32, bf16, f8, mult, sub, Tanh, variant=""):
    import concourse.mybir as mybir
    add = mybir.AluOpType.add
    wide = variant == "wide"
    n_oh = 1 if variant == "halfmm" else NOH
    if variant in ("mmonly", "mmrows256"):
        # diagnostic: pure PE stream — same matmult/psum-group structure as
        # the real kernel but constant operands, no DMA/recursion/drains.
        # Measures sustained matmult throughput on this hardware.
        nfree = 256 if variant == "mmrows256" else 512
        wt0 = wpool.tile([P, OUT_F], bf16, tag="w")
        nc.sync.dma_start(wt0[:], w[0, 0:P, :])
        for h in range(N_HALF):
            ps = [
                [
                    pspool.tile([P, nfree], f32, tag=f"psm_{bc}_{oh}",
                                name=f"psm_{bc}_{oh}")
                    for oh in range(NOH)
                ]
                for bc in range(NBC)
            ]
            for i in range(NI):
                for d in range(1, DEG + 1):
                    start = i == 0 and d == 1
                    stop = i == NI - 1 and d == DEG
                    for bc in range(NBC):
                        for oh in range(NOH):
                            nc.tensor.matmul(
                                ps[bc][oh],
                                ones,
                                wt0[:, oh * nfree : (oh + 1) * nfree],
                                start=start,
                                stop=stop,
                            )
        return
    for h in range(N_HALF):
            if wide:
                # one [P, 1024] tile spans 2 PSUM banks; a single matmult
                # accumulates the full 1024-wide output row block
                ps = [
                    pspool.tile([P, OUT_F], f32, tag=f"psw_{bc}",
                                name=f"psw_{bc}")
                    for bc in range(NBC)
                ]
            else:
                ps = [
                    [
                        pspool.tile(
                            [P, 512], f32, tag=f"ps_{bc}_{oh}",
                            name=f"ps_{bc}_{oh}"
                        )
                        for oh in range(n_oh)
                    ]
                    for bc in range(NBC)
                ]
            bm = variant == "bm"
            use_fp8 = variant in ("", "fp8")
            n_bf = 6 if use_fp8 else DEG  # degrees done in bf16
            ch8_list = []  # per k-tile pair j: {7: tile, 8: tile}
            for i in range(NI):
                last_i = i == NI - 1
                chds = {}
                wts = {}
                if use_fp8 and i % 2 == 0:
                    pair = {}
                    for dd in (DEG - 1, DEG):
                        pair[dd] = ch8pool.tile(
                            [P, 2, HALF], f8, tag=f"ch8_{dd}",
                            name=f"ch8_{h}_{i}_{dd}",
                        )
                    ch8_list.append(pair)
                xti = xpool.tile([P, HALF], f32, tag="x")
                nc.sync.dma_start(
                    xti[:], xt[i * P : (i + 1) * P, h * HALF : (h + 1) * HALF]
                )
                t = tanpool.tile([P, HALF], f32, tag="t")
                nc.scalar.activation(t[:], xti[:], Tanh)

                tm2 = None  # T_{d-2} (fp32); None encodes T_0 == 1
                tm1 = t  # T_{d-1} (fp32)
                ch1 = None
                for d in range(1, DEG + 1):
                    last = d == DEG
                    in_fp8 = use_fp8 and d > n_bf
                    if variant == "norec" and d > 1:
                        chd = ch1
                    elif in_fp8:
                        chd = None  # fp8 slot written instead (below)
                    else:
                        chd = chpool.tile([P, HALF], bf16, tag="ch")
                    if d == 1:
                        nc.scalar.copy(chd[:], t[:])
                        ch1 = chd
                        cur = t
                    elif variant == "norec":
                        cur = None
                    else:
                        # pr = (T_{d-1} * 2) * t  (one fused DVE op)
                        pr = rpool.tile([P, HALF], f32, tag="rec")
                        nc.vector.scalar_tensor_tensor(
                            pr[:], tm1[:], 2.0, t[:], mult, mult
                        )
                        if d == 2:
                            # T_2 = pr - 1
                            cur = rpool.tile([P, HALF], f32, tag="rec")
                            nc.vector.tensor_scalar_sub(cur[:], pr[:], 1.0)
                            nc.scalar.copy(chd[:], cur[:])
                        elif in_fp8:
                            # fp8 degree: keep the fp32 value for the
                            # recursion, cast into this k-tile pair's
                            # DoubleRow slot (slot = i parity)
                            cur = rpool.tile([P, HALF], f32, tag="rec")
                            nc.vector.tensor_tensor(cur[:], pr[:], tm2[:], sub)
                            nc.scalar.copy(
                                ch8_list[i // 2][d][:, i % 2, :], cur[:]
                            )
                            if last:
                                cur = None
                        elif not last:
                            cur = rpool.tile([P, HALF], f32, tag="rec")
                            nc.vector.tensor_tensor(cur[:], pr[:], tm2[:], sub)
                            nc.scalar.copy(chd[:], cur[:])
                        else:
                            # final degree: write the bf16 tile directly
                            cur = None
                            nc.vector.tensor_tensor(chd[:], pr[:], tm2[:], sub)
                    tm2, tm1 = tm1, cur

                    if in_fp8:
                        continue  # consumed by the DoubleRow wave below
                    if variant == "nodma":
                        if i == 0 and d == 1:
                            wt0 = wpool.tile([P, 1, OUT_F], bf16, tag="w")
                            nc.sync.dma_start(wt0[:, 0], w[0, 0:P, :])
                        wt = wt0[:, 0]
                    else:
                        wt = wpool.tile([P, OUT_F], bf16, tag="w")
                        nc.sync.dma_start(wt[:], w[d - 1, i * P : (i + 1) * P, :])
                    if bm and last_i:
                        # bank-major tail: defer the last k-tile's matmuls so
                        # they can be issued per-bank (below), letting each
                        # bank's drain start while later banks still stream
                        chds[d] = chd
                        wts[d] = wt
                        continue
                    stop = last_i and d == n_bf
                    start = i == 0 and d == 1
                    for bc in range(NBC):
                        if variant == "oneldw":
                            # diagnostic: constant stationary everywhere so
                            # the post-compile dedup strips nearly every
                            # ldweights; numerics wrong, timing isolates the
                            # ldweights contribution
                            lhsT = ones[:]
                        else:
                            lhsT = chd[:, bc * P : (bc + 1) * P]
                        if wide:
                            nc.tensor.matmul(
                                ps[bc], lhsT, wt[:], start=start, stop=stop
                            )
                        else:
                            for oh in range(n_oh):
                                nc.tensor.matmul(
                                    ps[bc][oh],
                                    lhsT,
                                    wt[:, oh * 512 : (oh + 1) * 512],
                                    start=start,
                                    stop=stop,
                                )
                if bm and last_i:
                    # last k-tile, bank-major: bank bc finishes all its
                    # matmuls early and its drain (with the d=0 bias add)
                    # is emitted immediately, so the DVE drains overlap the
                    # remaining banks' matmul tail instead of serializing
                    # at the half boundary in front of the next half's
                    # recursion ops on the in-order DVE queue.
                    for bc in range(NBC):
                        for d in range(1, DEG + 1):
                            lhsT = chds[d][:, bc * P : (bc + 1) * P]
                            for oh in range(n_oh):
                                nc.tensor.matmul(
                                    ps[bc][oh],
                                    lhsT,
                                    wts[d][:, oh * 512 : (oh + 1) * 512],
                                    start=False,
                                    stop=d == DEG,
                                )
                        if variant == "nodrain":
                            continue
                        for oh in range(n_oh):
                            ot = opool.tile([P, 512], f32, tag="ot")
                            nc.vector.tensor_tensor(
                                ot[:],
                                ps[bc][oh],
                                biasrep[:, oh * 512 : (oh + 1) * 512],
                                add,
                            )
                            r0 = h * HALF + bc * P
                            nc.sync.dma_start(
                                out[r0 : r0 + P, oh * 512 : (oh + 1) * 512],
                                ot[:],
                            )
            if bm:
                continue  # drains already emitted per-bank above
            if variant == "nodrain":
                continue
            if use_fp8:
                # Wave 1 (degrees 1..6, bf16) is complete: capture each
                # bank's partial + d=0 bias into SBUF, freeing the bank for
                # wave 2 (degrees 7..8 as fp8 e4m3 DoubleRow, coefficients
                # pre-scaled by 2^12 on the host; undone in the final merge).
                ots = {}
                for bc in range(NBC):
                    for oh in range(n_oh):
                        ot = opool.tile([P, 512], f32, tag="ot")
                        nc.vector.tensor_tensor(
                            ot[:], ps[bc][oh],
                            biasrep[:, oh * 512 : (oh + 1) * 512], add
                        )
                        ots[(bc, oh)] = ot
                for dd_idx, d in enumerate((DEG - 1, DEG)):
                    for j in range(NI // 2):
                        wt8 = w8pool.tile([P, 2, OUT_F], f8, tag="w8")
                        nc.sync.dma_start(wt8[:], w8[dd_idx, j])
                        start = dd_idx == 0 and j == 0
                        stop = dd_idx == 1 and j == NI // 2 - 1
                        for bc in range(NBC):
                            lhsT = ch8_list[j][d][:, :, bc * P : (bc + 1) * P]
                            for oh in range(n_oh):
                                nc.tensor.matmul(
                                    ps[bc][oh],
                                    lhsT,
                                    wt8[:, :, oh * 512 : (oh + 1) * 512],
                                    start=start,
                                    stop=stop,
                                    perf_mode=mybir.MatmulPerfMode.DoubleRow,
                                )
                # merge: out = wave2 * 2^-12 + (wave1 + bias), then store
                for bc in range(NBC):
                    for oh in range(n_oh):
                        ot2 = opool.tile([P, 512], f32, tag="ot2")
                        nc.vector.scalar_tensor_tensor(
                            ot2[:], ps[bc][oh], 2.0 ** -12,
                            ots[(bc, oh)][:], mult, add,
                        )
                        r0 = h * HALF + bc * P
                        nc.sync.dma_start(
                            out[r0 : r0 + P, oh * 512 : (oh + 1) * 512],
                            ot2[:],
                        )
                continue
            # Drain this half's PSUM to SBUF (adding the d=0 bias) and then
            # HBM. The bias-add rides the drain copy for free on DVE.
            for bc in range(NBC):
                for oh in range(n_oh):
                    ot = opool.tile([P, 512], f32, tag="ot")
                    src = (
                        ps[bc][:, oh * 512 : (oh + 1) * 512]
                        if wide
                        else ps[bc][oh]
                    )
                    nc.vector.tensor_tensor(
                        ot[:], src, biasrep[:, oh * 512 : (oh + 1) * 512], add
                    )
                    r0 = h * HALF + bc * P
                    nc.sync.dma_start(
                        out[r0 : r0 + P, oh * 512 : (oh + 1) * 512], ot[:]
                    )


def _get_nc(loop_r=None, variant=""):
    key = (loop_r, variant)
    if key not in _CACHED_NC:
        _CACHED_NC[key] = _build_bass(loop_r, variant)
    return _CACHED_NC[key]


def _prep_inputs(x, coefficients):
    bf16 = ml_dtypes.bfloat16
    e4 = ml_dtypes.float8_e4m3
    SC8 = 2.0 ** 12
    x = np.asarray(x, dtype=np.float32)
    coef = np.asarray(coefficients, dtype=np.float32)
    # (d, i, o) bf16 for d = 1..DEG (degrees 7..8 unused by the default
    # variant but kept so diagnostic variants stay runnable)
    w_all = np.ascontiguousarray(coef.transpose(2, 0, 1)[1 : DEG + 1]).astype(bf16)
    # degrees 7..8 in fp8 e4m3 (DoubleRow): k-tile pairs j=(2j, 2j+1) on a
    # separate axis, coefficients pre-scaled by 2^12 so they use e4m3's
    # normal range (sigma*2^12 ~ 0.44); the drain merge multiplies by 2^-12
    w8_arr = np.zeros((2, NI // 2, P, 2, OUT_F), dtype=e4)
    for dd, d in enumerate((DEG - 1, DEG)):
        for j in range(NI // 2):
            for s in range(2):
                blk = coef[(2 * j + s) * P : (2 * j + s + 1) * P, :, d] * SC8
                w8_arr[dd, j, :, s, :] = blk.astype(e4)
    # d=0 term is a per-output bias (T_0 == 1): summed over i on the host,
    # replicated across the 128 partitions, added during the PSUM drain
    bias = coef[:, :, 0].sum(axis=0, dtype=np.float64).astype(np.float32)
    brep_arr = np.ascontiguousarray(
        np.broadcast_to(bias, (P, OUT_F)), dtype=np.float32
    )
    in_maps = []
    for c in range(N_CORES):
        xc = x[c * B_CORE : (c + 1) * B_CORE, :]
        in_maps.append(
            {
                "xt": np.ascontiguousarray(xc.T),
                "w": w_all,
                "w8": w8_arr,
                "brep": brep_arr,
            }
        )
    return in_maps


VARIANT = ""  # production variant used by kernel()/run()


def run(x, coefficients, trace=False, tmpdir=None):
    """Run on hardware; returns (out, BassKernelResults)."""
    from concourse.bass_utils import run_bass_kernel_spmd

    nc = _get_nc(None, VARIANT)
    in_maps = _prep_inputs(x, coefficients)
    res = run_bass_kernel_spmd(
        nc,
        in_maps,
        core_ids=list(range(N_CORES)),
        trace=trace,
        tmpdir=tmpdir,
    )
    out = np.concatenate([r["out"] for r in res.results], axis=0)
    return np.ascontiguousarray(out, dtype=np.float32), res


def kernel(x, coefficients):
    out, _ = run(x, coefficients, trace=False)
    return out



# revision 28
# speedup vs baseline: 1.1877x; 1.0614x over previous
"""ChebyKAN linear layer on 8 Trainium2 NeuronCores.

Computation: out[b,o] = sum_{i,d} T_d(tanh(x[b,i])) * coef[i,o,d]
  == sum_d T_d(tanh(x)) @ C_d   (9 accumulated 8192x1024x1024 matmuls)

Strategy:
  - Data-parallel over batch: core c handles rows [c*1024, (c+1)*1024).
  - Host pre-transposes each core's x slice to (in_features, batch) layout so
    the contraction dim (i) lands on SBUF partitions, and repacks the
    coefficients to (d, i, o): bf16 for degrees 1..6, fp8 e4m3 (scaled by
    2^12, k-tile pairs on a separate axis) for degrees 7..8.
  - On-chip: ACT computes tanh in fp32, DVE runs the Chebyshev recursion
    T_d = 2 t T_{d-1} - T_{d-2} in fp32 (scalar_tensor_tensor fuses the
    2*t*T_{d-1} product into one op), ACT casts T_1..T_6 to bf16 and
    T_7..T_8 to fp8 e4m3 DoubleRow pair tiles [128, 2, 512].
  - PE accumulates per half in two PSUM waves: wave 1 = degrees 1..6 in
    bf16 (start at (i=0,d=1)); a DVE drain captures wave 1 + the d=0 bias
    (host-precomputed, fp32, partition-replicated) into SBUF; wave 2 =
    degrees 7..8 as fp8 DoubleRow matmuls (2 k-tiles per instruction, 2x
    MAC rate); the final DVE merge computes wave2 * 2^-12 + wave1 and DMAs
    out. Splitting waves keeps the 2^12 fp8 coefficient scale out of the
    bf16 accumulation.
  - The d=0 term rides the drain (no PE matmuls for it).
  - Per core the 1024-row batch is processed in two 512-column halves; each
    half keeps its full output (4 b-chunks x 2 o-halves) resident in all
    8 PSUM banks while the k-blocks accumulate into it.

Numerics (validated on HW): rel l2 error vs fp32 reference ~1.5e-2
(threshold 2e-2; fp8 on 2 of 8 degrees contributes ~sqrt(2)*1.04e-2, the
bf16 remainder ~1.8e-3).

Performance notes (8-core SPMD, measured via on-device For_i loop slope
because the axon tunnel's ~80-95 ms RPC overhead hides the kernel and NTFF
profiling is unavailable through it; cross-process slope noise is +-5 us):
  - Measured matmult throughput on this part (mmonly/mmrows256 diagnostic
    variants, pure PE stream, no deps) is 0.51 ns/row — 1.96 GHz effective,
    NOT the 2.4 GHz nominal — and is linear in rows with ~zero per-
    instruction overhead. The all-bf16 kernel (1024 512-row matmuls,
    267 us of stream) measured 269-283 us: already ~97% of the PE roofline.
  - Ldweights are nearly free when sync-free (hidden in the matmul
    pipeline): deduping 187 extra reloads moved the time < 2 us. The
    engine-aware dedup pass below keeps them at the 1-per-(bc-chunk) floor.
  - DMA (weights re-streamed per half, 33.6 MB/iter/core), PSUM drains and
    half boundaries are all second-order (nodma/nodrain variants within
    noise of base).
  - fp8 DoubleRow on degrees 7..8 removes 1/4 of the bf16 rows.
"""

import numpy as np
import ml_dtypes

BATCH = 8192
IN_F = 1024
OUT_F = 1024
DEG = 8  # degree; DEG+1 coefficients per (i,o)
FP8_DEGS = (6, 7, 8)  # degrees computed as fp8 e4m3 DoubleRow
N_CORES = 8
B_CORE = BATCH // N_CORES  # 1024
P = 128
HALF = 512  # batch columns processed per PSUM-resident output block
NI = IN_F // P  # 8 contraction tiles
NBC = HALF // P  # 4 b-chunks per half
NOH = OUT_F // 512  # 2 output halves of 512
N_HALF = B_CORE // HALF  # 2

_CACHED_NC = {}


def _build_bass(loop_r=None, variant=""):
    """Build the Bass program. loop_r wraps the whole compute in a hardware
    For loop of loop_r iterations (benchmark-only; slope over loop_r gives
    per-iteration HW time since the axon RPC overhead is per-call)."""
    import contextlib

    import concourse.mybir as mybir
    import concourse.tile as tile
    from concourse import bacc

    f32 = mybir.dt.float32
    bf16 = mybir.dt.bfloat16
    mult = mybir.AluOpType.mult
    sub = mybir.AluOpType.subtract
    Tanh = mybir.ActivationFunctionType.Tanh

    import json as _json

    def _dedup_ldweights(b):
        """Remove InstLdweights that reload the identical stationary operand
        still held by the PE array. Tile emits one Ldweights per matmul, so a
        weight reused by consecutive matmuls is loaded twice; each redundant
        load costs ~55 ns of serial PE time. Only sync-free exact duplicates
        are removed. Instructions on OTHER engine queues interleaved in the
        block's program order cannot disturb the PE weight registers, so they
        do not reset the tracked key; the only PE-queue instructions between
        a duplicate pair are Matmult (does not disturb weights) and sync-free
        EventSemaphore. An EventSemaphore with a wait is treated as a fence
        (conservative: it could order an SBUF rewrite of the stationary)."""
        n_removed = 0
        PE = mybir.EngineType.PE
        for fn in b.m.functions:
            for blk in fn.blocks:
                last_key = None
                keep = []
                for inst in blk.instructions:
                    if isinstance(inst, mybir.InstLdweights):
                        d = _json.loads(
                            mybir.instruction_to_pretty_json_string(inst)
                        )
                        si = d.get("sync_info") or {}
                        has_sync = bool(
                            si.get("on_wait") or si.get("on_update")
                        )
                        key = _json.dumps(
                            [
                                d.get("ins"),
                                d.get("perf_mode"),
                                d.get("is_transpose"),
                                d.get("tile_position"),
                                d.get("tile_size"),
                            ],
                            sort_keys=True,
                        )
                        if key == last_key and not has_sync:
                            n_removed += 1
                            continue
                        last_key = key
                    elif inst.engine != PE:
                        pass  # other-engine work; PE array unaffected
                    elif isinstance(inst, mybir.InstMatmult):
                        pass  # matmult does not disturb loaded weights
                    elif isinstance(inst, mybir.InstEventSemaphore):
                        si = inst.sync_info
                        if si is not None and si.on_wait:
                            # conservative fence: a PE-queue wait could
                            # order an SBUF rewrite of the stationary
                            last_key = None
                    else:
                        last_key = None
                    keep.append(inst)
                blk.instructions[:] = keep
        b._ldw_removed = n_removed

    class _Bacc(bacc.Bacc):
        def compile(self):
            super().compile()
            _dedup_ldweights(self)

    f8 = mybir.dt.float8e4

    nc = _Bacc(name="chebykan")
    xt = nc.dram_tensor("xt", (IN_F, B_CORE), f32, kind="ExternalInput")
    w = nc.dram_tensor("w", (DEG, IN_F, OUT_F), bf16, kind="ExternalInput")
    # degrees 7..8 as fp8 e4m3, k-tile pairs interleaved for DoubleRow:
    # w8[dd, j, k, s, o] = coef[(2j+s)*128 + k, o, 7+dd] * 2^12
    w8 = nc.dram_tensor(
        "w8", (len(FP8_DEGS), NI // 2, P, 2, OUT_F), f8, kind="ExternalInput"
    )
    brep = nc.dram_tensor("brep", (P, OUT_F), f32, kind="ExternalInput")
    out = nc.dram_tensor("out", (B_CORE, OUT_F), f32, kind="ExternalOutput")

    with (
        tile.TileContext(nc) as tc,
        tc.tile_pool(name="wpool", bufs=12) as wpool,
        tc.tile_pool(name="w8pool", bufs=6) as w8pool,
        tc.tile_pool(name="xpool", bufs=8) as xpool,
        tc.tile_pool(name="tanh", bufs=3) as tanpool,
        tc.tile_pool(name="rec", bufs=6) as rpool,
        tc.tile_pool(name="ch", bufs=16) as chpool,
        tc.tile_pool(name="ch8", bufs=16) as ch8pool,
        tc.tile_pool(name="const", bufs=1) as cpool,
        tc.tile_pool(name="outp", bufs=16) as opool,
        tc.tile_pool(name="psum", bufs=1, space="PSUM") as pspool,
    ):
        ones = cpool.tile([P, P], bf16)
        nc.vector.memset(ones[:], 1.0)
        # d=0 bias, pre-replicated across partitions on the host; added
        # during the PSUM drain instead of spending PE matmuls on it
        biasrep = cpool.tile([P, OUT_F], f32)
        nc.sync.dma_start(biasrep[:], brep[:, :])

        loop_cm = (
            tc.For_i(
                0,
                loop_r,
                1,
                hint_engines=(mybir.EngineType.PE, mybir.EngineType.SP),
            )
            if loop_r is not None
            else contextlib.nullcontext()
        )
        with loop_cm:
            _emit_body(nc, tc, xt, w, w8, out, ones, biasrep,
                       wpool, w8pool, xpool, tanpool, rpool, chpool, ch8pool,
                       opool, pspool, f32, bf16, f8, mult, sub, Tanh, variant)
    nc.finalize()
    return nc


def _emit_body(nc, tc, xt, w, w8, out, ones, biasrep,
               wpool, w8pool, xpool, tanpool, rpool, chpool, ch8pool,
               opool, pspool, f32, bf16, f8, mult, sub, Tanh, variant=""):
    import concourse.mybir as mybir
    add = mybir.AluOpType.add
    wide = variant == "wide"
    n_oh = 1 if variant == "halfmm" else NOH
    if variant in ("mmonly", "mmrows256"):
        # diagnostic: pure PE stream — same matmult/psum-group structure as
        # the real kernel but constant operands, no DMA/recursion/drains.
        # Measures sustained matmult throughput on this hardware.
        nfree = 256 if variant == "mmrows256" else 512
        wt0 = wpool.tile([P, OUT_F], bf16, tag="w")
        nc.sync.dma_start(wt0[:], w[0, 0:P, :])
        for h in range(N_HALF):
            ps = [
                [
                    pspool.tile([P, nfree], f32, tag=f"psm_{bc}_{oh}",
                                name=f"psm_{bc}_{oh}")
                    for oh in range(NOH)
                ]
                for bc in range(NBC)
            ]
            for i in range(NI):
                for d in range(1, DEG + 1):
                    start = i == 0 and d == 1
                    stop = i == NI - 1 and d == DEG
                    for bc in range(NBC):
                        for oh in range(NOH):
                            nc.tensor.matmul(
                                ps[bc][oh],
                                ones,
                                wt0[:, oh * nfree : (oh + 1) * nfree],
                                start=start,
                                stop=stop,
                            )
        return
    for h in range(N_HALF):
            if wide:
                # one [P, 1024] tile spans 2 PSUM banks; a single matmult
                # accumulates the full 1024-wide output row block
                ps = [
                    pspool.tile([P, OUT_F], f32, tag=f"psw_{bc}",
                                name=f"psw_{bc}")
                    for bc in range(NBC)
                ]
            else:
                ps = [
                    [
                        pspool.tile(
                            [P, 512], f32, tag=f"ps_{bc}_{oh}",
                            name=f"ps_{bc}_{oh}"
                        )
                        for oh in range(n_oh)
                    ]
                    for bc in range(NBC)
                ]
            bm = variant == "bm"
            use_fp8 = variant in ("", "fp8")
            n_bf = DEG - len(FP8_DEGS) if use_fp8 else DEG  # degrees in bf16
            ch8_list = []  # per k-tile pair j: {7: tile, 8: tile}
            for i in range(NI):
                last_i = i == NI - 1
                chds = {}
                wts = {}
                if use_fp8 and i % 2 == 0:
                    pair = {}
                    for dd in FP8_DEGS:
                        pair[dd] = ch8pool.tile(
                            [P, 2, HALF], f8, tag=f"ch8_{dd}",
                            name=f"ch8_{h}_{i}_{dd}",
                        )
                    ch8_list.append(pair)
                xti = xpool.tile([P, HALF], f32, tag="x")
                nc.sync.dma_start(
                    xti[:], xt[i * P : (i + 1) * P, h * HALF : (h + 1) * HALF]
                )
                t = tanpool.tile([P, HALF], f32, tag="t")
                nc.scalar.activation(t[:], xti[:], Tanh)

                tm2 = None  # T_{d-2} (fp32); None encodes T_0 == 1
                tm1 = t  # T_{d-1} (fp32)
                ch1 = None
                for d in range(1, DEG + 1):
                    last = d == DEG
                    in_fp8 = use_fp8 and d > n_bf
                    if variant == "norec" and d > 1:
                        chd = ch1
                    elif in_fp8:
                        chd = None  # fp8 slot written instead (below)
                    else:
                        chd = chpool.tile([P, HALF], bf16, tag="ch")
                    if d == 1:
                        nc.scalar.copy(chd[:], t[:])
                        ch1 = chd
                        cur = t
                    elif variant == "norec":
                        cur = None
                    else:
                        # pr = (T_{d-1} * 2) * t  (one fused DVE op)
                        pr = rpool.tile([P, HALF], f32, tag="rec")
                        nc.vector.scalar_tensor_tensor(
                            pr[:], tm1[:], 2.0, t[:], mult, mult
                        )
                        if d == 2:
                            # T_2 = pr - 1
                            cur = rpool.tile([P, HALF], f32, tag="rec")
                            nc.vector.tensor_scalar_sub(cur[:], pr[:], 1.0)
                            nc.scalar.copy(chd[:], cur[:])
                        elif in_fp8:
                            # fp8 degree: keep the fp32 value for the
                            # recursion, cast into this k-tile pair's
                            # DoubleRow slot (slot = i parity)
                            cur = rpool.tile([P, HALF], f32, tag="rec")
                            nc.vector.tensor_tensor(cur[:], pr[:], tm2[:], sub)
                            nc.scalar.copy(
                                ch8_list[i // 2][d][:, i % 2, :], cur[:]
                            )
                            if last:
                                cur = None
                        elif not last:
                            cur = rpool.tile([P, HALF], f32, tag="rec")
                            nc.vector.tensor_tensor(cur[:], pr[:], tm2[:], sub)
                            nc.scalar.copy(chd[:], cur[:])
                        else:
                            # final degree: write the bf16 tile directly
                            cur = None
                            nc.vector.tensor_tensor(chd[:], pr[:], tm2[:], sub)
                    tm2, tm1 = tm1, cur

                    if in_fp8:
                        continue  # consumed by the DoubleRow wave below
                    if variant == "nodma":
                        if i == 0 and d == 1:
                            wt0 = wpool.tile([P, 1, OUT_F], bf16, tag="w")
                            nc.sync.dma_start(wt0[:, 0], w[0, 0:P, :])
                        wt = wt0[:, 0]
                    else:
                        wt = wpool.tile([P, OUT_F], bf16, tag="w")
                        nc.sync.dma_start(wt[:], w[d - 1, i * P : (i + 1) * P, :])
                    if bm and last_i:
                        # bank-major tail: defer the last k-tile's matmuls so
                        # they can be issued per-bank (below), letting each
                        # bank's drain start while later banks still stream
                        chds[d] = chd
                        wts[d] = wt
                        continue
                    stop = last_i and d == n_bf
                    start = i == 0 and d == 1
                    for bc in range(NBC):
                        if variant == "oneldw":
                            # diagnostic: constant stationary everywhere so
                            # the post-compile dedup strips nearly every
                            # ldweights; numerics wrong, timing isolates the
                            # ldweights contribution
                            lhsT = ones[:]
                        else:
                            lhsT = chd[:, bc * P : (bc + 1) * P]
                        if wide:
                            nc.tensor.matmul(
                                ps[bc], lhsT, wt[:], start=start, stop=stop
                            )
                        else:
                            for oh in range(n_oh):
                                nc.tensor.matmul(
                                    ps[bc][oh],
                                    lhsT,
                                    wt[:, oh * 512 : (oh + 1) * 512],
                                    start=start,
                                    stop=stop,
                                )
                if bm and last_i:
                    # last k-tile, bank-major: bank bc finishes all its
                    # matmuls early and its drain (with the d=0 bias add)
                    # is emitted immediately, so the DVE drains overlap the
                    # remaining banks' matmul tail instead of serializing
                    # at the half boundary in front of the next half's
                    # recursion ops on the in-order DVE queue.
                    for bc in range(NBC):
                        for d in range(1, DEG + 1):
                            lhsT = chds[d][:, bc * P : (bc + 1) * P]
                            for oh in range(n_oh):
                                nc.tensor.matmul(
                                    ps[bc][oh],
                                    lhsT,
                                    wts[d][:, oh * 512 : (oh + 1) * 512],
                                    start=False,
                                    stop=d == DEG,
                                )
                        if variant == "nodrain":
                            continue
                        for oh in range(n_oh):
                            ot = opool.tile([P, 512], f32, tag="ot")
                            nc.vector.tensor_tensor(
                                ot[:],
                                ps[bc][oh],
                                biasrep[:, oh * 512 : (oh + 1) * 512],
                                add,
                            )
                            r0 = h * HALF + bc * P
                            nc.sync.dma_start(
                                out[r0 : r0 + P, oh * 512 : (oh + 1) * 512],
                                ot[:],
                            )
            if bm:
                continue  # drains already emitted per-bank above
            if variant == "nodrain":
                continue
            if use_fp8:
                # Wave 1 (degrees 1..6, bf16) is complete: capture each
                # bank's partial + d=0 bias into SBUF, freeing the bank for
                # wave 2 (degrees 7..8 as fp8 e4m3 DoubleRow, coefficients
                # pre-scaled by 2^12 on the host; undone in the final merge).
                ots = {}
                for bc in range(NBC):
                    for oh in range(n_oh):
                        ot = opool.tile([P, 512], f32, tag="ot")
                        nc.vector.tensor_tensor(
                            ot[:], ps[bc][oh],
                            biasrep[:, oh * 512 : (oh + 1) * 512], add
                        )
                        ots[(bc, oh)] = ot
                for dd_idx, d in enumerate(FP8_DEGS):
                    for j in range(NI // 2):
                        wt8 = w8pool.tile([P, 2, OUT_F], f8, tag="w8")
                        nc.sync.dma_start(wt8[:], w8[dd_idx, j])
                        start = dd_idx == 0 and j == 0
                        stop = dd_idx == len(FP8_DEGS) - 1 and j == NI // 2 - 1
                        for bc in range(NBC):
                            lhsT = ch8_list[j][d][:, :, bc * P : (bc + 1) * P]
                            for oh in range(n_oh):
                                nc.tensor.matmul(
                                    ps[bc][oh],
                                    lhsT,
                                    wt8[:, :, oh * 512 : (oh + 1) * 512],
                                    start=start,
                                    stop=stop,
                                    perf_mode=mybir.MatmulPerfMode.DoubleRow,
                                )
                # merge: out = wave2 * 2^-12 + (wave1 + bias), then store
                for bc in range(NBC):
                    for oh in range(n_oh):
                        ot2 = opool.tile([P, 512], f32, tag="ot2")
                        nc.vector.scalar_tensor_tensor(
                            ot2[:], ps[bc][oh], 2.0 ** -12,
                            ots[(bc, oh)][:], mult, add,
                        )
                        r0 = h * HALF + bc * P
                        nc.sync.dma_start(
                            out[r0 : r0 + P, oh * 512 : (oh + 1) * 512],
                            ot2[:],
                        )
                continue
            # Drain this half's PSUM to SBUF (adding the d=0 bias) and then
            # HBM. The bias-add rides the drain copy for free on DVE.
            for bc in range(NBC):
                for oh in range(n_oh):
                    ot = opool.tile([P, 512], f32, tag="ot")
                    src = (
                        ps[bc][:, oh * 512 : (oh + 1) * 512]
                        if wide
                        else ps[bc][oh]
                    )
                    nc.vector.tensor_tensor(
                        ot[:], src, biasrep[:, oh * 512 : (oh + 1) * 512], add
                    )
                    r0 = h * HALF + bc * P
                    nc.sync.dma_start(
                        out[r0 : r0 + P, oh * 512 : (oh + 1) * 512], ot[:]
                    )


def _get_nc(loop_r=None, variant=""):
    key = (loop_r, variant)
    if key not in _CACHED_NC:
        _CACHED_NC[key] = _build_bass(loop_r, variant)
    return _CACHED_NC[key]


def _prep_inputs(x, coefficients):
    bf16 = ml_dtypes.bfloat16
    e4 = ml_dtypes.float8_e4m3
    SC8 = 2.0 ** 12
    x = np.asarray(x, dtype=np.float32)
    coef = np.asarray(coefficients, dtype=np.float32)
    # (d, i, o) bf16 for d = 1..DEG (degrees 7..8 unused by the default
    # variant but kept so diagnostic variants stay runnable)
    w_all = np.ascontiguousarray(coef.transpose(2, 0, 1)[1 : DEG + 1]).astype(bf16)
    # degrees 7..8 in fp8 e4m3 (DoubleRow): k-tile pairs j=(2j, 2j+1) on a
    # separate axis, coefficients pre-scaled by 2^12 so they use e4m3's
    # normal range (sigma*2^12 ~ 0.44); the drain merge multiplies by 2^-12
    w8_arr = np.zeros((len(FP8_DEGS), NI // 2, P, 2, OUT_F), dtype=e4)
    for dd, d in enumerate(FP8_DEGS):
        for j in range(NI // 2):
            for s in range(2):
                blk = coef[(2 * j + s) * P : (2 * j + s + 1) * P, :, d] * SC8
                w8_arr[dd, j, :, s, :] = blk.astype(e4)
    # d=0 term is a per-output bias (T_0 == 1): summed over i on the host,
    # replicated across the 128 partitions, added during the PSUM drain
    bias = coef[:, :, 0].sum(axis=0, dtype=np.float64).astype(np.float32)
    brep_arr = np.ascontiguousarray(
        np.broadcast_to(bias, (P, OUT_F)), dtype=np.float32
    )
    in_maps = []
    for c in range(N_CORES):
        xc = x[c * B_CORE : (c + 1) * B_CORE, :]
        in_maps.append(
            {
                "xt": np.ascontiguousarray(xc.T),
                "w": w_all,
                "w8": w8_arr,
                "brep": brep_arr,
            }
        )
    return in_maps


VARIANT = ""  # production variant used by kernel()/run()


def run(x, coefficients, trace=False, tmpdir=None):
    """Run on hardware; returns (out, BassKernelResults)."""
    from concourse.bass_utils import run_bass_kernel_spmd

    nc = _get_nc(None, VARIANT)
    in_maps = _prep_inputs(x, coefficients)
    res = run_bass_kernel_spmd(
        nc,
        in_maps,
        core_ids=list(range(N_CORES)),
        trace=trace,
        tmpdir=tmpdir,
    )
    out = np.concatenate([r["out"] for r in res.results], axis=0)
    return np.ascontiguousarray(out, dtype=np.float32), res


def kernel(x, coefficients):
    out, _ = run(x, coefficients, trace=False)
    return out

